# revision 33
# baseline (speedup 1.0000x reference)
"""Poincare fully-connected (hyperbolic linear) forward on 8 TRN2 NeuronCores.

Wire-optimized v3. The axon tunnel runs at ~60-120 MB/s shared, so the call
is bandwidth-bound on host<->device transfers, not device compute. Strategy:

- Send x as raw fp16 in its natural [B, 64] layout (134 MB total vs 335 MB
  for the old bf16 hi/lo augmented layout); return the output quantized to
  i8 via round(out*127) (67 MB fetch + 67 MB donated zero-init buffers,
  ~3.9e-3 max abs error on outputs bounded by the unit ball -- gate 2e-2).
  Host work: one astype(f16) pass in, one np.multiply dequant pass out.
- Everything else on device: lam = 2/(1-|x|^2) via square+reduce, x
  transposed one 64-row subtile at a time on the PE array (identity
  matmul; operands stay at partition base 0 -- base-64 after a transpose
  wedges the PE), then t = lam*(x @ z2) - (lam-1)*sinh(2b) assembled as
  R*(mm - sh2) + sinh with R = 1/(1-|x|^2), z2 = z*2cosh(2b)/||z||
  (f16 hi + lo accumulated). asinh/sinh tail via ln/exp tables only.
- The batch loop is a hardware For_i (body = one 2048-row chunk, laid out
  [128 partitions, 16 rows, 64 feats], 2 KB contiguous DMA lines): the
  bass_exec XLA hook re-runs walrus on every call (fresh jit closure, no
  NEFF cache on that path), so a ~64x smaller BIR cuts ~0.7 s/call.

Data-parallel over batch: 131072 rows/core on 8 cores.
"""
import os
import numpy as np
from contextlib import ExitStack

import concourse.bass as bass
import concourse.bacc as bacc
import concourse.tile as tile
import concourse.mybir as mybir
import concourse.masks as masks
from concourse.bass_utils import run_bass_kernel_spmd

f32 = np.float32
f16 = np.float16

B, IN, OUT = 1048576, 64, 64
NCORES = 8
BC = B // NCORES            # rows per core
CHUNK = 2048                # batch rows per chunk
SUB = CHUNK // 128          # 16 rows per partition per chunk
NCHUNK = BC // CHUNK        # 64

AF = mybir.ActivationFunctionType
ALU = mybir.AluOpType

LAST_RESULTS = None
LAST_WALL = None


def _build_nc():
    nc = bacc.Bacc("TRN2", target_bir_lowering=False, debug=False,
                   enable_asserts=False, num_devices=NCORES)
    x16 = nc.dram_tensor("x16", [BC, IN], mybir.dt.float16, kind="ExternalInput").ap()
    z2h = nc.dram_tensor("z2h", [IN, OUT], mybir.dt.float16, kind="ExternalInput").ap()
    z2l = nc.dram_tensor("z2l", [IN, OUT], mybir.dt.float16, kind="ExternalInput").ap()
    cst = nc.dram_tensor("cst", [128, 3, OUT], mybir.dt.float32, kind="ExternalInput").ap()
    o8 = nc.dram_tensor("o8", [BC, OUT], mybir.dt.int8, kind="ExternalOutput").ap()
    # row = cp*SUB + s with cp = c*128 + p: a hardware loop over cp blocks of
    # 128 partitions keeps the BIR ~64x smaller than full unrolling, which
    # matters because the bass_exec XLA hook re-runs walrus on EVERY call
    xv = x16.rearrange("(cp s) d -> cp (s d)", s=SUB)
    ov = o8.rearrange("(cp s) d -> cp (s d)", s=SUB)

    with tile.TileContext(nc) as tc, \
         tc.tile_pool(name="const", bufs=1) as cpool, \
         tc.tile_pool(name="io", bufs=2) as iopool, \
         tc.tile_pool(name="xt", bufs=2) as xtpool, \
         tc.tile_pool(name="work", bufs=2) as wpool, \
         tc.tile_pool(name="small", bufs=2) as spool, \
         tc.tile_pool(name="psum", bufs=2, space="PSUM") as ppool, \
         tc.tile_pool(name="psumtr", bufs=2, space="PSUM") as tpool:
        z2h_t = cpool.tile([IN, OUT], mybir.dt.float16, tag="z2h")
        z2l_t = cpool.tile([IN, OUT], mybir.dt.float16, tag="z2l")
        cst_t = cpool.tile([128, 3, OUT], mybir.dt.float32, tag="cst")
        ident = cpool.tile([128, 128], mybir.dt.float16, tag="id")
        nc.sync.dma_start(z2h_t[:], z2h)
        nc.sync.dma_start(z2l_t[:], z2l)
        nc.sync.dma_start(cst_t[:], cst)
        masks.make_identity(nc, ident[:])

        with tc.For_i(0, BC // SUB, 128) as cp0:
            xt = iopool.tile([128, SUB, IN], mybir.dt.float16, tag="x")
            nc.sync.dma_start(xt[:], xv[bass.ds(cp0, 128), :])

            # R = 1 / (1 - sum_d x^2) = lam / 2
            xsq = iopool.tile([128, SUB, IN], mybir.dt.float16, tag="xsq")
            nc.vector.tensor_tensor(xsq[:], xt[:], xt[:], ALU.mult)
            s1 = spool.tile([128, SUB, 1], mybir.dt.float32, tag="s1")
            nc.vector.tensor_reduce(s1[:], xsq[:], axis=mybir.AxisListType.X, op=ALU.add)
            om = spool.tile([128, SUB, 1], mybir.dt.float32, tag="om")
            nc.vector.tensor_scalar(om[:], s1[:], -1.0, 1.0, ALU.mult, ALU.add)
            R = spool.tile([128, SUB, 1], mybir.dt.float32, tag="R")
            nc.vector.reciprocal(R[:], om[:])

            # mm[p, s, j] = sum_d x[p, s, d] * z2[d, j] via PE-array transpose.
            # One 64-row subtile per transpose: keeps every matmul operand at
            # partition base 0 (a transpose followed by a base-64 stationary
            # load wedges the PE).
            tp = ppool.tile([128, SUB, OUT], mybir.dt.float32, tag="t")
            for s in range(SUB):
                tr = tpool.tile([64, 128], mybir.dt.float16, tag="tr")
                nc.tensor.transpose(tr[:], xt[:, s, :], ident[:])
                xT = xtpool.tile([64, 128], mybir.dt.float16, tag="xT")
                nc.scalar.activation(xT[:], tr[:], AF.Copy)
                nc.tensor.matmul(tp[:, s, :], xT[:], z2h_t[:],
                                 start=True, stop=False)
                nc.tensor.matmul(tp[:, s, :], xT[:], z2l_t[:],
                                 start=False, stop=True)

            # arg = R*(mm - sh2) + sinh
            _, sh2b = bass.broadcast_tensor_aps(tp[:], cst_t[:, 0:1, :])
            a1 = wpool.tile([128, SUB, OUT], mybir.dt.float32, tag="A")
            nc.vector.tensor_tensor(a1[:], tp[:], sh2b, ALU.subtract)
            _, Rb = bass.broadcast_tensor_aps(a1[:], R[:])
            a2 = wpool.tile([128, SUB, OUT], mybir.dt.float32, tag="B")
            nc.vector.tensor_tensor(a2[:], a1[:], Rb, ALU.mult)
            _, sinb = bass.broadcast_tensor_aps(a2[:], cst_t[:, 1:2, :])
            arg = wpool.tile([128, SUB, OUT], mybir.dt.float32, tag="C")
            nc.gpsimd.tensor_tensor(arg[:], a2[:], sinb, ALU.add)

            # L = asinh(arg) = ln(arg + sqrt(1 + arg^2)), ln/exp tables only
            t2 = wpool.tile([128, SUB, OUT], mybir.dt.float32, tag="D")
            nc.scalar.activation(t2[:], arg[:], AF.Square)
            g = wpool.tile([128, SUB, OUT], mybir.dt.float32, tag="A")
            nc.scalar.activation(g[:], t2[:], AF.Ln, bias=1.0)
            sq = wpool.tile([128, SUB, OUT], mybir.dt.float32, tag="B")
            nc.scalar.activation(sq[:], g[:], AF.Exp, scale=0.5)
            u = wpool.tile([128, SUB, OUT], mybir.dt.float32, tag="D")
            nc.vector.tensor_tensor(u[:], arg[:], sq[:], ALU.add)
            L = wpool.tile([128, SUB, OUT], mybir.dt.float32, tag="A")
            nc.scalar.activation(L[:], u[:], AF.Ln)

            # w2 = 2*sinh(k2*L) = e^(k2*L) - e^(-k2*L)
            _, k2b = bass.broadcast_tensor_aps(L[:], cst_t[:, 2:3, :])
            L2 = wpool.tile([128, SUB, OUT], mybir.dt.float32, tag="B")
            nc.vector.tensor_tensor(L2[:], L[:], k2b, ALU.mult)
            e1 = wpool.tile([128, SUB, OUT], mybir.dt.float32, tag="C")
            nc.scalar.activation(e1[:], L2[:], AF.Exp)
            ei = wpool.tile([128, SUB, OUT], mybir.dt.float32, tag="D")
            nc.scalar.activation(ei[:], L2[:], AF.Exp, scale=-1.0)
            w2 = wpool.tile([128, SUB, OUT], mybir.dt.float32, tag="A")
            nc.vector.tensor_tensor(w2[:], e1[:], ei[:], ALU.subtract)

            # out = w2 / (2 + sqrt(4 + sum_j w2^2))
            wsq = wpool.tile([128, SUB, OUT], mybir.dt.float32, tag="B")
            nc.gpsimd.tensor_tensor(wsq[:], w2[:], w2[:], ALU.mult)
            ss = spool.tile([128, SUB, 1], mybir.dt.float32, tag="ss")
            nc.vector.tensor_reduce(ss[:], wsq[:], axis=mybir.AxisListType.X, op=ALU.add)
            ss4 = spool.tile([128, SUB, 1], mybir.dt.float32, tag="ss4")
            nc.vector.tensor_scalar_add(ss4[:], ss[:], 4.0)
            q = spool.tile([128, SUB, 1], mybir.dt.float32, tag="q")
            nc.scalar.activation(q[:], ss4[:], AF.Ln)
            dd = spool.tile([128, SUB, 1], mybir.dt.float32, tag="dd")
            nc.scalar.activation(dd[:], q[:], AF.Exp, scale=0.5)
            d2 = spool.tile([128, SUB, 1], mybir.dt.float32, tag="d2")
            nc.vector.tensor_scalar_add(d2[:], dd[:], 2.0)
            r = spool.tile([128, SUB, 1], mybir.dt.float32, tag="r")
            nc.vector.reciprocal(r[:], d2[:])

            otf = wpool.tile([128, SUB, OUT], mybir.dt.float32, tag="C")
            _, rb = bass.broadcast_tensor_aps(w2[:], r[:])
            nc.vector.tensor_tensor(otf[:], w2[:], rb, ALU.mult)
            # quantize out in (-1,1) to i8: round(out*127); DVE float->int
            # conversion is round-to-nearest-even, and i8 makes the host
            # dequant a single multiply pass
            ot = iopool.tile([128, SUB, OUT], mybir.dt.int8, tag="o")
            nc.vector.tensor_scalar(ot[:], otf[:], 127.0, None, ALU.mult)
            nc.sync.dma_start(ov[bass.ds(cp0, 128), :], ot[:])
    nc.compile()
    return nc


_NC_CACHE = None


LAST_PHASES = None


def _consts(z, bias):
    z64 = np.asarray(z, np.float64)
    b64 = np.asarray(bias, np.float64)
    z_norm = np.maximum(np.linalg.norm(z64, axis=0), 1e-15)
    cosh2 = np.cosh(2.0 * b64)
    sinh2 = np.sinh(2.0 * b64)
    z2 = z64 * (2.0 * cosh2 / z_norm)[None, :]
    z2h = z2.astype(f16)
    z2l = (z2 - z2h.astype(np.float64)).astype(f16)
    cst = np.empty((128, 3, OUT), f32)
    cst[:, 0, :] = 2.0 * sinh2
    cst[:, 1, :] = sinh2
    cst[:, 2, :] = 2.0 * z_norm
    return z2h, z2l, cst


def _dequant_into(res, out_slice, nrows):
    # res.results[c]["o8"] are views into one cached [nrows, OUT] u8 fetch;
    # dequantize straight into the caller's f32 slice (2 vectorized passes)
    base = res.results[0]["o8"].base
    if isinstance(base, np.ndarray) and base.shape == (nrows, OUT) and base.dtype == np.int8:
        q8 = base
    else:
        q8 = np.concatenate([r["o8"] for r in res.results], axis=0)
    np.multiply(q8, f32(1.0 / 127.0), dtype=f32, out=out_slice)


def kernel(x: np.ndarray, z: np.ndarray, bias: np.ndarray) -> np.ndarray:
    global _NC_CACHE, LAST_RESULTS, LAST_WALL, LAST_PHASES
    import time
    tA = time.time()
    x = np.asarray(x, f32)
    z2h, z2l, cst = _consts(z, bias)

    if _NC_CACHE is None:
        _NC_CACHE = _build_nc()
    nc = _NC_CACHE
    os.environ["BASS_NEVER_TRACE"] = "1"  # no NTFF hook in this container

    x16 = x.astype(f16)
    in_maps = [{
        "x16": x16[cid * BC:(cid + 1) * BC],
        "z2h": z2h, "z2l": z2l, "cst": cst,
    } for cid in range(NCORES)]
    # prefault the 268 MB result buffer while the tunnel transfer blocks on
    # IO (the fill releases the GIL), so the dequant write takes no faults
    import threading
    out = np.empty((B, OUT), f32)
    pf = threading.Thread(target=out.fill, args=(0.0,))
    pf.start()
    t0 = time.time()
    res = run_bass_kernel_spmd(nc, in_maps, list(range(NCORES)), trace=False)
    LAST_WALL = time.time() - t0
    LAST_RESULTS = res
    pf.join()
    _dequant_into(res, out, B)
    t2 = time.time()
    LAST_PHASES = {"pre": t0 - tA, "spmd": LAST_WALL, "post": t2 - t0 - LAST_WALL}
    return out


# revision 34
# speedup vs baseline: 1.1876x; 1.1876x over previous
"""Poincare fully-connected (hyperbolic linear) forward on 8 TRN2 NeuronCores.

Wire-optimized v3. The axon tunnel runs at ~60-120 MB/s shared, so the call
is bandwidth-bound on host<->device transfers, not device compute. Strategy:

- Send x as raw fp16 in its natural [B, 64] layout (134 MB total vs 335 MB
  for the old bf16 hi/lo augmented layout); return the output quantized to
  i8 via round(out*127) (67 MB fetch + 67 MB donated zero-init buffers,
  ~3.9e-3 max abs error on outputs bounded by the unit ball -- gate 2e-2).
  Host work: one astype(f16) pass in, one np.multiply dequant pass out.
- Everything else on device: lam = 2/(1-|x|^2) via square+reduce, x
  transposed one 64-row subtile at a time on the PE array (identity
  matmul; operands stay at partition base 0 -- base-64 after a transpose
  wedges the PE), then t = lam*(x @ z2) - (lam-1)*sinh(2b) assembled as
  R*(mm - sh2) + sinh with R = 1/(1-|x|^2), z2 = z*2cosh(2b)/||z||
  (f16 hi + lo accumulated). asinh/sinh tail via ln/exp tables only.
- The batch loop is a hardware For_i (body = one 2048-row chunk, laid out
  [128 partitions, 16 rows, 64 feats], 2 KB contiguous DMA lines): the
  bass_exec XLA hook re-runs walrus on every call (fresh jit closure, no
  NEFF cache on that path), so a ~64x smaller BIR cuts ~0.7 s/call.

Data-parallel over batch: 131072 rows/core on 8 cores.
"""
import os
import numpy as np
from contextlib import ExitStack

import hashlib

import concourse.bass as bass
import concourse.bacc as bacc
import concourse.tile as tile
import concourse.mybir as mybir
import concourse.masks as masks
import concourse.bass2jax as bass2jax
from concourse.bass_utils import run_bass_kernel_spmd

# The bass_exec XLA hook re-runs the full walrus BIR->NEFF compile on every
# call (each call jits a fresh closure; the stock jit path consults the
# libneuronxla NEFF cache, this path doesn't). compile_bir_kernel is a
# deterministic function of the BIR bytes, so memoize it per-process the
# same way libneuronxla memoizes stock modules -- the NEFF that executes is
# bit-identical, only the redundant recompile is skipped.
_NEFF_MEMO: dict = {}
_ORIG_COMPILE_BIR = bass2jax.compile_bir_kernel


def _memo_compile_bir_kernel(bir_json, tmpdir, neff_name="file.neff"):
    raw = bir_json if isinstance(bir_json, bytes) else bir_json.encode()
    key = (hashlib.sha256(raw).hexdigest(), neff_name)
    data = _NEFF_MEMO.get(key)
    if data is None:
        path = _ORIG_COMPILE_BIR(bir_json, tmpdir, neff_name=neff_name)
        with open(path, "rb") as f:
            _NEFF_MEMO[key] = f.read()
        return path
    path = os.path.join(tmpdir, neff_name)
    with open(path, "wb") as f:
        f.write(data)
    return path


bass2jax.compile_bir_kernel = _memo_compile_bir_kernel

f32 = np.float32
f16 = np.float16

B, IN, OUT = 1048576, 64, 64
NCORES = 8
BC = B // NCORES            # rows per core
CHUNK = 2048                # batch rows per chunk
SUB = CHUNK // 128          # 16 rows per partition per chunk
NCHUNK = BC // CHUNK        # 64

AF = mybir.ActivationFunctionType
ALU = mybir.AluOpType

LAST_RESULTS = None
LAST_WALL = None


def _build_nc():
    nc = bacc.Bacc("TRN2", target_bir_lowering=False, debug=False,
                   enable_asserts=False, num_devices=NCORES)
    x16 = nc.dram_tensor("x16", [BC, IN], mybir.dt.float16, kind="ExternalInput").ap()
    z2h = nc.dram_tensor("z2h", [IN, OUT], mybir.dt.float16, kind="ExternalInput").ap()
    z2l = nc.dram_tensor("z2l", [IN, OUT], mybir.dt.float16, kind="ExternalInput").ap()
    cst = nc.dram_tensor("cst", [128, 3, OUT], mybir.dt.float32, kind="ExternalInput").ap()
    o8 = nc.dram_tensor("o8", [BC, OUT], mybir.dt.int8, kind="ExternalOutput").ap()
    # row = cp*SUB + s with cp = c*128 + p: a hardware loop over cp blocks of
    # 128 partitions keeps the BIR ~64x smaller than full unrolling, which
    # matters because the bass_exec XLA hook re-runs walrus on EVERY call
    xv = x16.rearrange("(cp s) d -> cp (s d)", s=SUB)
    ov = o8.rearrange("(cp s) d -> cp (s d)", s=SUB)

    with tile.TileContext(nc) as tc, \
         tc.tile_pool(name="const", bufs=1) as cpool, \
         tc.tile_pool(name="io", bufs=2) as iopool, \
         tc.tile_pool(name="xt", bufs=2) as xtpool, \
         tc.tile_pool(name="work", bufs=2) as wpool, \
         tc.tile_pool(name="small", bufs=2) as spool, \
         tc.tile_pool(name="psum", bufs=2, space="PSUM") as ppool, \
         tc.tile_pool(name="psumtr", bufs=2, space="PSUM") as tpool:
        z2h_t = cpool.tile([IN, OUT], mybir.dt.float16, tag="z2h")
        z2l_t = cpool.tile([IN, OUT], mybir.dt.float16, tag="z2l")
        cst_t = cpool.tile([128, 3, OUT], mybir.dt.float32, tag="cst")
        ident = cpool.tile([128, 128], mybir.dt.float16, tag="id")
        nc.sync.dma_start(z2h_t[:], z2h)
        nc.sync.dma_start(z2l_t[:], z2l)
        nc.sync.dma_start(cst_t[:], cst)
        masks.make_identity(nc, ident[:])

        with tc.For_i(0, BC // SUB, 128) as cp0:
            xt = iopool.tile([128, SUB, IN], mybir.dt.float16, tag="x")
            nc.sync.dma_start(xt[:], xv[bass.ds(cp0, 128), :])

            # R = 1 / (1 - sum_d x^2) = lam / 2
            xsq = iopool.tile([128, SUB, IN], mybir.dt.float16, tag="xsq")
            nc.vector.tensor_tensor(xsq[:], xt[:], xt[:], ALU.mult)
            s1 = spool.tile([128, SUB, 1], mybir.dt.float32, tag="s1")
            nc.vector.tensor_reduce(s1[:], xsq[:], axis=mybir.AxisListType.X, op=ALU.add)
            om = spool.tile([128, SUB, 1], mybir.dt.float32, tag="om")
            nc.vector.tensor_scalar(om[:], s1[:], -1.0, 1.0, ALU.mult, ALU.add)
            R = spool.tile([128, SUB, 1], mybir.dt.float32, tag="R")
            nc.vector.reciprocal(R[:], om[:])

            # mm[p, s, j] = sum_d x[p, s, d] * z2[d, j] via PE-array transpose.
            # One 64-row subtile per transpose: keeps every matmul operand at
            # partition base 0 (a transpose followed by a base-64 stationary
            # load wedges the PE).
            tp = ppool.tile([128, SUB, OUT], mybir.dt.float32, tag="t")
            for s in range(SUB):
                tr = tpool.tile([64, 128], mybir.dt.float16, tag="tr")
                nc.tensor.transpose(tr[:], xt[:, s, :], ident[:])
                xT = xtpool.tile([64, 128], mybir.dt.float16, tag="xT")
                nc.scalar.activation(xT[:], tr[:], AF.Copy)
                nc.tensor.matmul(tp[:, s, :], xT[:], z2h_t[:],
                                 start=True, stop=False)
                nc.tensor.matmul(tp[:, s, :], xT[:], z2l_t[:],
                                 start=False, stop=True)

            # arg = R*(mm - sh2) + sinh
            _, sh2b = bass.broadcast_tensor_aps(tp[:], cst_t[:, 0:1, :])
            a1 = wpool.tile([128, SUB, OUT], mybir.dt.float32, tag="A")
            nc.vector.tensor_tensor(a1[:], tp[:], sh2b, ALU.subtract)
            _, Rb = bass.broadcast_tensor_aps(a1[:], R[:])
            a2 = wpool.tile([128, SUB, OUT], mybir.dt.float32, tag="B")
            nc.vector.tensor_tensor(a2[:], a1[:], Rb, ALU.mult)
            _, sinb = bass.broadcast_tensor_aps(a2[:], cst_t[:, 1:2, :])
            arg = wpool.tile([128, SUB, OUT], mybir.dt.float32, tag="C")
            nc.gpsimd.tensor_tensor(arg[:], a2[:], sinb, ALU.add)

            # L = asinh(arg) = ln(arg + sqrt(1 + arg^2)), ln/exp tables only
            t2 = wpool.tile([128, SUB, OUT], mybir.dt.float32, tag="D")
            nc.scalar.activation(t2[:], arg[:], AF.Square)
            g = wpool.tile([128, SUB, OUT], mybir.dt.float32, tag="A")
            nc.scalar.activation(g[:], t2[:], AF.Ln, bias=1.0)
            sq = wpool.tile([128, SUB, OUT], mybir.dt.float32, tag="B")
            nc.scalar.activation(sq[:], g[:], AF.Exp, scale=0.5)
            u = wpool.tile([128, SUB, OUT], mybir.dt.float32, tag="D")
            nc.vector.tensor_tensor(u[:], arg[:], sq[:], ALU.add)
            L = wpool.tile([128, SUB, OUT], mybir.dt.float32, tag="A")
            nc.scalar.activation(L[:], u[:], AF.Ln)

            # w2 = 2*sinh(k2*L) = e^(k2*L) - e^(-k2*L)
            _, k2b = bass.broadcast_tensor_aps(L[:], cst_t[:, 2:3, :])
            L2 = wpool.tile([128, SUB, OUT], mybir.dt.float32, tag="B")
            nc.vector.tensor_tensor(L2[:], L[:], k2b, ALU.mult)
            e1 = wpool.tile([128, SUB, OUT], mybir.dt.float32, tag="C")
            nc.scalar.activation(e1[:], L2[:], AF.Exp)
            ei = wpool.tile([128, SUB, OUT], mybir.dt.float32, tag="D")
            nc.scalar.activation(ei[:], L2[:], AF.Exp, scale=-1.0)
            w2 = wpool.tile([128, SUB, OUT], mybir.dt.float32, tag="A")
            nc.vector.tensor_tensor(w2[:], e1[:], ei[:], ALU.subtract)

            # out = w2 / (2 + sqrt(4 + sum_j w2^2))
            wsq = wpool.tile([128, SUB, OUT], mybir.dt.float32, tag="B")
            nc.gpsimd.tensor_tensor(wsq[:], w2[:], w2[:], ALU.mult)
            ss = spool.tile([128, SUB, 1], mybir.dt.float32, tag="ss")
            nc.vector.tensor_reduce(ss[:], wsq[:], axis=mybir.AxisListType.X, op=ALU.add)
            ss4 = spool.tile([128, SUB, 1], mybir.dt.float32, tag="ss4")
            nc.vector.tensor_scalar_add(ss4[:], ss[:], 4.0)
            q = spool.tile([128, SUB, 1], mybir.dt.float32, tag="q")
            nc.scalar.activation(q[:], ss4[:], AF.Ln)
            dd = spool.tile([128, SUB, 1], mybir.dt.float32, tag="dd")
            nc.scalar.activation(dd[:], q[:], AF.Exp, scale=0.5)
            d2 = spool.tile([128, SUB, 1], mybir.dt.float32, tag="d2")
            nc.vector.tensor_scalar_add(d2[:], dd[:], 2.0)
            r = spool.tile([128, SUB, 1], mybir.dt.float32, tag="r")
            nc.vector.reciprocal(r[:], d2[:])

            otf = wpool.tile([128, SUB, OUT], mybir.dt.float32, tag="C")
            _, rb = bass.broadcast_tensor_aps(w2[:], r[:])
            nc.vector.tensor_tensor(otf[:], w2[:], rb, ALU.mult)
            # quantize out in (-1,1) to i8: round(out*127); DVE float->int
            # conversion is round-to-nearest-even, and i8 makes the host
            # dequant a single multiply pass
            ot = iopool.tile([128, SUB, OUT], mybir.dt.int8, tag="o")
            nc.vector.tensor_scalar(ot[:], otf[:], 127.0, None, ALU.mult)
            nc.sync.dma_start(ov[bass.ds(cp0, 128), :], ot[:])
    nc.compile()
    return nc


_NC_CACHE = None


LAST_PHASES = None


def _consts(z, bias):
    z64 = np.asarray(z, np.float64)
    b64 = np.asarray(bias, np.float64)
    z_norm = np.maximum(np.linalg.norm(z64, axis=0), 1e-15)
    cosh2 = np.cosh(2.0 * b64)
    sinh2 = np.sinh(2.0 * b64)
    z2 = z64 * (2.0 * cosh2 / z_norm)[None, :]
    z2h = z2.astype(f16)
    z2l = (z2 - z2h.astype(np.float64)).astype(f16)
    cst = np.empty((128, 3, OUT), f32)
    cst[:, 0, :] = 2.0 * sinh2
    cst[:, 1, :] = sinh2
    cst[:, 2, :] = 2.0 * z_norm
    return z2h, z2l, cst


def _dequant_into(res, out_slice, nrows):
    # res.results[c]["o8"] are views into one cached [nrows, OUT] u8 fetch;
    # dequantize straight into the caller's f32 slice (2 vectorized passes)
    base = res.results[0]["o8"].base
    if isinstance(base, np.ndarray) and base.shape == (nrows, OUT) and base.dtype == np.int8:
        q8 = base
    else:
        q8 = np.concatenate([r["o8"] for r in res.results], axis=0)
    np.multiply(q8, f32(1.0 / 127.0), dtype=f32, out=out_slice)


def kernel(x: np.ndarray, z: np.ndarray, bias: np.ndarray) -> np.ndarray:
    global _NC_CACHE, LAST_RESULTS, LAST_WALL, LAST_PHASES
    import time
    tA = time.time()
    x = np.asarray(x, f32)
    z2h, z2l, cst = _consts(z, bias)

    if _NC_CACHE is None:
        _NC_CACHE = _build_nc()
    nc = _NC_CACHE
    os.environ["BASS_NEVER_TRACE"] = "1"  # no NTFF hook in this container

    x16 = x.astype(f16)
    in_maps = [{
        "x16": x16[cid * BC:(cid + 1) * BC],
        "z2h": z2h, "z2l": z2l, "cst": cst,
    } for cid in range(NCORES)]
    # prefault the 268 MB result buffer while the tunnel transfer blocks on
    # IO (the fill releases the GIL), so the dequant write takes no faults
    import threading
    out = np.empty((B, OUT), f32)
    pf = threading.Thread(target=out.fill, args=(0.0,))
    pf.start()
    t0 = time.time()
    res = run_bass_kernel_spmd(nc, in_maps, list(range(NCORES)), trace=False)
    LAST_WALL = time.time() - t0
    LAST_RESULTS = res
    pf.join()
    _dequant_into(res, out, B)
    t2 = time.time()
    LAST_PHASES = {"pre": t0 - tA, "spmd": LAST_WALL, "post": t2 - t0 - LAST_WALL}
    return out
